# revision 4
# baseline (speedup 1.0000x reference)
"""Trainium2 Bass kernel for nn_BEMBFlex (within-category log-softmax utility).

Sharding: items dealt by category across 8 cores (categories rank-sorted by
size, rank % 8 -> shard), one SPMD program for all cores. Each core computes
util for all 1024 sessions over its ~1/8 of items, then the within-category
log-softmax locally.

Layout (the key trick vs the previous version): within each block, item
columns are POSITION-MAJOR: col = t * g + q for slot q < g, within-category
position t < L. Consequences:
  - segment sums become a cascade of CONTIGUOUS bf16 adds (DVE 2x mode),
  - the final log-prob op is an int16 TT subtract whose lsc operand is a
    4D AP [part, [256,8], [0,L'], [1,g]] with stride-1 last dim -> 2x mode
    (the old slot-major broadcast AP had a stride-0 last dim -> stuck at 1x),
  - all 8 session-chunks of a block are processed by ONE DVE op via a
    chunk-stride AP dim, amortizing instruction overhead 8x.

Output is in log-bits scale: out_i16 = bits16(ex) - bits16(s), where
ln(x) ~ bits16(x)*ln2/128 - 127*ln2 (biases cancel in the subtraction).
Host multiplies by K16 = ln2/128 during the de-permute.

Blocks are sized g*L <= 512 so one PSUM bank-aligned 512-region holds one
(block, chunk) pair; a [P, 2048] PSUM tile holds 4 chunks and is drained by
a single ScalarE Exp. PE streams 2 matmuls (util + rank-1 lambda) per
region; back-to-back streaming reaches the full-speed PE p-state.
"""

import sys

for _p in ("/opt/trn_rl_repo",):
    if _p not in sys.path:
        sys.path.insert(0, _p)

import ml_dtypes
import numpy as np

import concourse.bass as bass
import concourse.tile as tile
from concourse import bacc, bass_utils, mybir

NUM_USERS = 100000
NUM_ITEMS = 25000
NUM_CATS = 500
LATENT = 64
BATCH = 1024
NCORES = 8
P = 128
NCHUNKS = BATCH // P
REGION = 512          # psum bank-aligned region = one (block, chunk)
CSTR_EX = 512         # ex tile per-chunk stride
CSTR_TR = 256         # tree scratch per-chunk stride
PAD_NEG = -1.0e30
SHIFT = 18.0
LN2 = float(np.log(2.0))
K16 = LN2 / (1 << 7)  # bf16-bit -> ln scale

F32 = mybir.dt.float32
BF16 = mybir.dt.bfloat16
I16 = mybir.dt.int16

_nc_cache = {}


# ----------------------------------------------------------------------------
# Host-side layout
# ----------------------------------------------------------------------------

def _layout(cat_sizes):
    """Blocks of slots with uniform tree width L, g*L <= REGION.

    Categories sorted by size desc; slot i holds ranks [8i, 8i+8) (one per
    shard). L = max size in the leading slot rounded up to even; g = how many
    slots fit in a 512 region. Lp = max true size in the block (final/out
    cover only positions t < Lp).
    """
    order = np.argsort(-cat_sizes, kind="stable")
    order = order[cat_sizes[order] > 0]
    ncats = len(order)
    nslots = -(-ncats // NCORES)
    slot_max = np.empty(nslots, np.int64)
    for i in range(nslots):
        slot_max[i] = int(cat_sizes[order[i * NCORES]])
    blocks = []  # (slot0, g, L, Lp)
    i = 0
    while i < nslots:
        Lp = int(slot_max[i])
        L = Lp + (Lp & 1)
        L = max(L, 2)
        g = min(REGION // L, nslots - i)
        blocks.append((i, g, L, Lp))
        i += g
    return order, blocks


def _prep(inputs):
    cat = np.asarray(inputs["category_idx"]).astype(np.int64).ravel()
    cat_sizes = np.bincount(cat, minlength=NUM_CATS)
    order, blocks = _layout(cat_sizes)
    nb = len(blocks)

    rank = np.full(NUM_CATS, -1, np.int64)
    rank[order] = np.arange(len(order))

    perm = np.argsort(cat, kind="stable")
    starts = np.searchsorted(cat[perm], np.arange(NUM_CATS))
    within_sorted = np.arange(NUM_ITEMS) - starts[cat[perm]]
    item_within = np.empty(NUM_ITEMS, np.int64)
    item_within[perm] = within_sorted

    # per-slot -> block index, q, and per-block col bases
    nslots = -(-len(order) // NCORES)
    blk_of_slot = np.empty(nslots, np.int64)
    q_of_slot = np.empty(nslots, np.int64)
    g_of_slot = np.empty(nslots, np.int64)
    ob0 = np.empty(nb, np.int64)
    acc = 0
    for b, (s0, g, L, Lp) in enumerate(blocks):
        blk_of_slot[s0:s0 + g] = b
        q_of_slot[s0:s0 + g] = np.arange(g)
        g_of_slot[s0:s0 + g] = g
        ob0[b] = acc
        acc += g * Lp
    opad = acc

    r = rank[cat]
    slot = r // NCORES
    item_shard = r % NCORES
    blk = blk_of_slot[slot]
    item_wcol = blk * REGION + item_within * g_of_slot[slot] + q_of_slot[slot]
    item_ocol = ob0[blk] + item_within * g_of_slot[slot] + q_of_slot[slot]

    alpha = np.ascontiguousarray(np.asarray(inputs["alpha_item"], np.float32))
    obs = np.ascontiguousarray(np.asarray(inputs["item_obs"], np.float32))
    lam = np.asarray(inputs["lambda_item"], np.float32).ravel()

    wpad = nb * REGION
    W = np.zeros((NCORES, 2 * LATENT, wpad), np.float32)
    LAMS = np.full((NCORES, 1, wpad), PAD_NEG, np.float32)
    for s in range(NCORES):
        m = item_shard == s
        cols = item_wcol[m]
        W[s, 0:LATENT, cols] = alpha[m]
        W[s, LATENT:, cols] = obs[m]
        LAMS[s, 0, cols] = lam[m] - SHIFT
    W = W.astype(ml_dtypes.bfloat16)
    LAMS = LAMS.astype(ml_dtypes.bfloat16)

    uidx = np.asarray(inputs["user_index"]).astype(np.int64).ravel()
    theta = np.asarray(inputs["theta_user"], np.float32)
    zeta = np.asarray(inputs["zeta_user"], np.float32)
    thzet = np.ascontiguousarray(
        np.concatenate([theta[uidx], zeta[uidx]], axis=1).T
    ).astype(ml_dtypes.bfloat16)
    return {
        "blocks": blocks,
        "opad": opad,
        "item_shard": item_shard,
        "item_ocol": item_ocol,
        "W": W,
        "LAMS": LAMS,
        "thzet": thzet,
    }


# ----------------------------------------------------------------------------
# Device program
# ----------------------------------------------------------------------------

def _ap3(t2d, off, cstr, n, w):
    """[P, N] tile -> [P, n, w] AP: chunk-stride cstr, packed inner width."""
    ap = t2d[:, :]
    return bass.AP(tensor=ap.tensor, offset=ap.offset + off,
                   ap=[ap.ap[0], [cstr, n], [1, w]])


def _ap4(t2d, off, cstr, n, rep, w):
    """[P, N] tile -> [P, n, rep, w] AP with a step-0 middle dim."""
    ap = t2d[:, :]
    return bass.AP(tensor=ap.tensor, offset=ap.offset + off,
                   ap=[ap.ap[0], [cstr, n], [0, rep], [1, w]])


def _build_nc(blocks, opad):
    nb = len(blocks)
    wpad = nb * REGION
    nc = bacc.Bacc(
        "TRN2",
        debug=False,
        enable_asserts=False,
        target_bir_lowering=False,
        num_devices=NCORES,
    )
    w_d = nc.dram_tensor("W", [2 * LATENT, wpad], BF16, kind="ExternalInput").ap()
    lams_d = nc.dram_tensor("LAMS", [1, wpad], BF16, kind="ExternalInput").ap()
    thzet_d = nc.dram_tensor("THZET", [2 * LATENT, BATCH], BF16,
                             kind="ExternalInput").ap()
    out_d = nc.dram_tensor("O", [BATCH, opad], I16, kind="ExternalOutput").ap()

    with tile.TileContext(nc) as tc:
        with (
            tc.tile_pool(name="singles", bufs=1) as singles,
            tc.tile_pool(name="psum_u", bufs=2, space="PSUM") as psum_u,
            tc.tile_pool(name="exbuf", bufs=3) as exbuf,
            tc.tile_pool(name="treebuf", bufs=3) as treebuf,
            tc.tile_pool(name="obuf", bufs=3) as obuf,
        ):
            thzet_sb = singles.tile([2 * LATENT, BATCH], BF16, name="thzet_sb")
            nc.sync.dma_start(out=thzet_sb[:, :], in_=thzet_d[:, :])
            ones_sb = singles.tile([1, P], BF16, name="ones_sb")
            nc.vector.memset(ones_sb[:, :], 1.0)
            thze_t = [thzet_sb[:, c * P:(c + 1) * P] for c in range(NCHUNKS)]
            w_sb = singles.tile([2 * LATENT, wpad], BF16, name="w_sb")
            lams_sb = singles.tile([1, wpad], BF16, name="lams_sb")
            # gpsimd HWDGE: 25ns issue (vs 565-784 on sync/scalar rings)
            nc.gpsimd.dma_start(out=lams_sb[:, :], in_=lams_d[:, :])
            nc.gpsimd.dma_start(out=w_sb[:, 0:REGION], in_=w_d[:, 0:REGION])
            nc.gpsimd.dma_start(out=w_sb[:, REGION:], in_=w_d[:, REGION:])

            ob0 = 0
            for b, (s0, g, L, Lp) in enumerate(blocks):
                wc0 = b * REGION
                cols = g * L
                ex = exbuf.tile([P, NCHUNKS * CSTR_EX], BF16, name="ex", tag="ex")
                # PE + ScalarE: two tiles of 4 chunk-regions each
                for half in range(2):
                    up = psum_u.tile([P, 4 * REGION], F32, name="up", tag="up")
                    for ci in range(4):
                        c = half * 4 + ci
                        nc.tensor.matmul(
                            up[:, ci * REGION:(ci + 1) * REGION],
                            lhsT=thze_t[c],
                            rhs=w_sb[:, wc0:wc0 + REGION],
                            start=True, stop=False,
                        )
                    for ci in range(4):
                        nc.tensor.matmul(
                            up[:, ci * REGION:(ci + 1) * REGION],
                            lhsT=ones_sb[0:1, :],
                            rhs=lams_sb[0:1, wc0:wc0 + REGION],
                            start=False, stop=True,
                        )
                    nc.scalar.activation(
                        out=ex[:, half * 4 * CSTR_EX:(half + 1) * 4 * CSTR_EX],
                        in_=up[:, 0:4 * REGION],
                        func=mybir.ActivationFunctionType.Exp,
                    )
                # DVE: cascade of contiguous adds across all 8 chunks at once
                trA = treebuf.tile([P, NCHUNKS * CSTR_TR], BF16,
                                   name="trA", tag="trA")
                trB = treebuf.tile([P, NCHUNKS * CSTR_TR], BF16,
                                   name="trB", tag="trB")
                h = L // 2
                nc.vector.tensor_add(
                    out=_ap3(trA, 0, CSTR_TR, NCHUNKS, g * h),
                    in0=_ap3(ex, 0, CSTR_EX, NCHUNKS, g * h),
                    in1=_ap3(ex, g * h, CSTR_EX, NCHUNKS, g * h),
                )
                w_, cur, nxt = h, trA, trB
                while w_ > 1:
                    hc = (w_ + 1) // 2
                    hh = w_ - hc
                    nc.vector.tensor_add(
                        out=_ap3(nxt, 0, CSTR_TR, NCHUNKS, g * hh),
                        in0=_ap3(cur, 0, CSTR_TR, NCHUNKS, g * hh),
                        in1=_ap3(cur, g * hc, CSTR_TR, NCHUNKS, g * hh),
                    )
                    if hc > hh:  # odd width: middle column passes through
                        nc.vector.tensor_copy(
                            out=_ap3(nxt, g * hh, CSTR_TR, NCHUNKS, g),
                            in_=_ap3(cur, g * hh, CSTR_TR, NCHUNKS, g),
                        )
                    w_, cur, nxt = hc, nxt, cur
                # s (bf16) sits at offset 0 of each chunk segment of `cur`
                ob = obuf.tile([P, NCHUNKS * g * Lp], I16, name="ob", tag="ob")
                nc.vector.tensor_tensor(
                    out=_ap3(ob, 0, g * Lp, NCHUNKS, g * Lp),
                    in0=_ap3(ex, 0, CSTR_EX, NCHUNKS, g * Lp).bitcast(I16),
                    in1=_ap4(cur, 0, CSTR_TR, NCHUNKS, Lp, g).bitcast(I16),
                    op=mybir.AluOpType.subtract,
                )
                for c in range(NCHUNKS):
                    nc.gpsimd.dma_start(
                        out=out_d[c * P:(c + 1) * P, ob0:ob0 + g * Lp],
                        in_=ob[:, c * g * Lp:(c + 1) * g * Lp],
                    )
                ob0 += g * Lp
    nc.compile()
    return nc


# ----------------------------------------------------------------------------
# Entry points
# ----------------------------------------------------------------------------

def run(inputs, trace=False):
    prep = _prep(inputs)
    key = (prep["opad"], tuple(prep["blocks"]))
    nc = _nc_cache.get(key)
    if nc is None:
        print(f"[kernel] opad={prep['opad']} nb={len(prep['blocks'])} "
              f"blocks={prep['blocks']}", file=sys.stderr)
        nc = _build_nc(prep["blocks"], prep["opad"])
        _nc_cache[key] = nc
    in_maps = [
        {
            "W": prep["W"][c],
            "LAMS": prep["LAMS"][c],
            "THZET": prep["thzet"],
        }
        for c in range(NCORES)
    ]
    res = bass_utils.run_bass_kernel_spmd(
        nc, in_maps, core_ids=list(range(NCORES)), trace=trace
    )
    big = np.stack(
        [np.asarray(res.results[c]["O"]) for c in range(NCORES)]
    )  # [8, B, opad] i16
    out = np.ascontiguousarray(
        big[prep["item_shard"], :, prep["item_ocol"]].T
    ).astype(np.float32) * np.float32(K16)
    return out, res


def kernel(**inputs) -> np.ndarray:
    out, _ = run(inputs, trace=False)
    return out


# revision 6
# speedup vs baseline: 1.0519x; 1.0519x over previous
"""Trainium2 Bass kernel for nn_BEMBFlex (within-category log-softmax utility).

Sharding: items dealt by category across 8 cores (categories rank-sorted by
size, rank % 8 -> shard), one SPMD program for all cores. Each core computes
util for all 1024 sessions over its ~1/8 of items, then the within-category
log-softmax locally.

Layout (the key trick vs the previous version): within each block, item
columns are POSITION-MAJOR: col = t * g + q for slot q < g, within-category
position t < L. Consequences:
  - segment sums become a cascade of CONTIGUOUS bf16 adds (DVE 2x mode),
  - the final log-prob op is an int16 TT subtract whose lsc operand is a
    4D AP [part, [256,8], [0,L'], [1,g]] with stride-1 last dim -> 2x mode
    (the old slot-major broadcast AP had a stride-0 last dim -> stuck at 1x),
  - all 8 session-chunks of a block are processed by ONE DVE op via a
    chunk-stride AP dim, amortizing instruction overhead 8x.

Output is in log-bits scale: out_i16 = bits16(ex) - bits16(s), where
ln(x) ~ bits16(x)*ln2/128 - 127*ln2 (biases cancel in the subtraction).
Host multiplies by K16 = ln2/128 during the de-permute.

Blocks are sized g*L <= 512 so one PSUM bank-aligned 512-region holds one
(block, chunk) pair; a [P, 2048] PSUM tile holds 4 chunks and is drained by
a single ScalarE Exp. PE streams 2 matmuls (util + rank-1 lambda) per
region; back-to-back streaming reaches the full-speed PE p-state.
"""

import sys

for _p in ("/opt/trn_rl_repo",):
    if _p not in sys.path:
        sys.path.insert(0, _p)

import ml_dtypes
import numpy as np

import concourse.bass as bass
import concourse.tile as tile
from concourse import bacc, bass_utils, mybir

NUM_USERS = 100000
NUM_ITEMS = 25000
NUM_CATS = 500
LATENT = 64
BATCH = 1024
NCORES = 8
P = 128
NCHUNKS = BATCH // P
REGION = 512          # psum bank-aligned region = one (block, chunk)
CSTR_EX = 512         # ex tile per-chunk stride
CSTR_TR = 256         # tree scratch per-chunk stride
PAD_NEG = -1.0e30
SHIFT = 18.0
LN2 = float(np.log(2.0))
K16 = LN2 / (1 << 7)  # bf16-bit -> ln scale

F32 = mybir.dt.float32
BF16 = mybir.dt.bfloat16
I16 = mybir.dt.int16

_nc_cache = {}


# ----------------------------------------------------------------------------
# Host-side layout
# ----------------------------------------------------------------------------

def _layout(cat_sizes):
    """Blocks of slots with uniform tree width L, g*L <= REGION.

    Categories sorted by size desc; slot i holds ranks [8i, 8i+8) (one per
    shard). L = max size in the leading slot rounded up to even; g = how many
    slots fit in a 512 region. Lp = max true size in the block (final/out
    cover only positions t < Lp).
    """
    order = np.argsort(-cat_sizes, kind="stable")
    order = order[cat_sizes[order] > 0]
    ncats = len(order)
    nslots = -(-ncats // NCORES)
    slot_max = np.empty(nslots, np.int64)
    for i in range(nslots):
        slot_max[i] = int(cat_sizes[order[i * NCORES]])
    blocks = []  # (slot0, g, L, Lp)
    i = 0
    while i < nslots:
        Lp = int(slot_max[i])
        L = Lp + (Lp & 1)
        L = max(L, 2)
        g = min(REGION // L, nslots - i)
        blocks.append((i, g, L, Lp))
        i += g
    return order, blocks


def _prep(inputs):
    cat = np.asarray(inputs["category_idx"]).astype(np.int64).ravel()
    cat_sizes = np.bincount(cat, minlength=NUM_CATS)
    order, blocks = _layout(cat_sizes)
    nb = len(blocks)

    rank = np.full(NUM_CATS, -1, np.int64)
    rank[order] = np.arange(len(order))

    perm = np.argsort(cat, kind="stable")
    starts = np.searchsorted(cat[perm], np.arange(NUM_CATS))
    within_sorted = np.arange(NUM_ITEMS) - starts[cat[perm]]
    item_within = np.empty(NUM_ITEMS, np.int64)
    item_within[perm] = within_sorted

    # per-slot -> block index, q, and per-block col bases
    nslots = -(-len(order) // NCORES)
    blk_of_slot = np.empty(nslots, np.int64)
    q_of_slot = np.empty(nslots, np.int64)
    g_of_slot = np.empty(nslots, np.int64)
    ob0 = np.empty(nb, np.int64)
    acc = 0
    for b, (s0, g, L, Lp) in enumerate(blocks):
        blk_of_slot[s0:s0 + g] = b
        q_of_slot[s0:s0 + g] = np.arange(g)
        g_of_slot[s0:s0 + g] = g
        ob0[b] = acc
        acc += g * Lp
    opad = acc

    r = rank[cat]
    slot = r // NCORES
    item_shard = r % NCORES
    blk = blk_of_slot[slot]
    item_wcol = blk * REGION + item_within * g_of_slot[slot] + q_of_slot[slot]
    item_ocol = ob0[blk] + item_within * g_of_slot[slot] + q_of_slot[slot]

    alpha = np.ascontiguousarray(np.asarray(inputs["alpha_item"], np.float32))
    obs = np.ascontiguousarray(np.asarray(inputs["item_obs"], np.float32))
    lam = np.asarray(inputs["lambda_item"], np.float32).ravel()

    wpad = nb * REGION
    W = np.zeros((NCORES, 2 * LATENT, wpad), np.float32)
    LAMS = np.full((NCORES, 1, wpad), PAD_NEG, np.float32)
    for s in range(NCORES):
        m = item_shard == s
        cols = item_wcol[m]
        W[s, 0:LATENT, cols] = alpha[m]
        W[s, LATENT:, cols] = obs[m]
        LAMS[s, 0, cols] = lam[m] - SHIFT
    W = W.astype(ml_dtypes.bfloat16)
    LAMS = LAMS.astype(ml_dtypes.bfloat16)

    uidx = np.asarray(inputs["user_index"]).astype(np.int64).ravel()
    theta = np.asarray(inputs["theta_user"], np.float32)
    zeta = np.asarray(inputs["zeta_user"], np.float32)
    thzet = np.ascontiguousarray(
        np.concatenate([theta[uidx], zeta[uidx]], axis=1).T
    ).astype(ml_dtypes.bfloat16)
    return {
        "blocks": blocks,
        "opad": opad,
        "item_shard": item_shard,
        "item_ocol": item_ocol,
        "W": W,
        "LAMS": LAMS,
        "thzet": thzet,
    }


# ----------------------------------------------------------------------------
# Device program
# ----------------------------------------------------------------------------

def _ap3(t2d, off, cstr, n, w):
    """[P, N] tile -> [P, n, w] AP: chunk-stride cstr, packed inner width."""
    ap = t2d[:, :]
    return bass.AP(tensor=ap.tensor, offset=ap.offset + off,
                   ap=[ap.ap[0], [cstr, n], [1, w]])


def _ap4(t2d, off, cstr, n, rep, w):
    """[P, N] tile -> [P, n, rep, w] AP with a step-0 middle dim."""
    ap = t2d[:, :]
    return bass.AP(tensor=ap.tensor, offset=ap.offset + off,
                   ap=[ap.ap[0], [cstr, n], [0, rep], [1, w]])


def _build_nc(blocks, opad):
    nb = len(blocks)
    wpad = nb * REGION
    nc = bacc.Bacc(
        "TRN2",
        debug=False,
        enable_asserts=False,
        target_bir_lowering=False,
        num_devices=NCORES,
    )
    w_d = nc.dram_tensor("W", [2 * LATENT, wpad], BF16, kind="ExternalInput").ap()
    lams_d = nc.dram_tensor("LAMS", [1, wpad], BF16, kind="ExternalInput").ap()
    thzet_d = nc.dram_tensor("THZET", [2 * LATENT, BATCH], BF16,
                             kind="ExternalInput").ap()
    out_d = nc.dram_tensor("O", [BATCH, opad], I16, kind="ExternalOutput").ap()

    with tile.TileContext(nc) as tc:
        with (
            tc.tile_pool(name="singles", bufs=1) as singles,
            tc.tile_pool(name="psum_u", bufs=2, space="PSUM") as psum_u,
            tc.tile_pool(name="exbuf", bufs=3) as exbuf,
            tc.tile_pool(name="treebuf", bufs=3) as treebuf,
            tc.tile_pool(name="obuf", bufs=3) as obuf,
        ):
            thzet_sb = singles.tile([2 * LATENT, BATCH], BF16, name="thzet_sb")
            nc.sync.dma_start(out=thzet_sb[:, :], in_=thzet_d[:, :])
            ones_sb = singles.tile([1, P], BF16, name="ones_sb")
            nc.vector.memset(ones_sb[:, :], 1.0)
            thze_t = [thzet_sb[:, c * P:(c + 1) * P] for c in range(NCHUNKS)]
            w_sb = singles.tile([2 * LATENT, wpad], BF16, name="w_sb")
            lams_sb = singles.tile([1, wpad], BF16, name="lams_sb")
            # gpsimd HWDGE: 25ns issue (vs 565-784 on sync/scalar rings)
            nc.gpsimd.dma_start(out=lams_sb[:, :], in_=lams_d[:, :])
            nc.gpsimd.dma_start(out=w_sb[:, 0:REGION], in_=w_d[:, 0:REGION])
            nc.gpsimd.dma_start(out=w_sb[:, REGION:], in_=w_d[:, REGION:])

            ob0 = 0
            for b, (s0, g, L, Lp) in enumerate(blocks):
                wc0 = b * REGION
                cols = g * L
                ex = exbuf.tile([P, NCHUNKS * CSTR_EX], BF16, name="ex", tag="ex")
                # PE + ScalarE: two tiles of 4 chunk-regions each
                for half in range(2):
                    up = psum_u.tile([P, 4 * REGION], F32, name="up", tag="up")
                    for ci in range(4):
                        c = half * 4 + ci
                        nc.tensor.matmul(
                            up[:, ci * REGION:(ci + 1) * REGION],
                            lhsT=thze_t[c],
                            rhs=w_sb[:, wc0:wc0 + REGION],
                            start=True, stop=False,
                        )
                    for ci in range(4):
                        nc.tensor.matmul(
                            up[:, ci * REGION:(ci + 1) * REGION],
                            lhsT=ones_sb[0:1, :],
                            rhs=lams_sb[0:1, wc0:wc0 + REGION],
                            start=False, stop=True,
                        )
                    nc.scalar.activation(
                        out=ex[:, half * 4 * CSTR_EX:(half + 1) * 4 * CSTR_EX],
                        in_=up[:, 0:4 * REGION],
                        func=mybir.ActivationFunctionType.Exp,
                    )
                # DVE: cascade of contiguous adds across all 8 chunks at once
                trA = treebuf.tile([P, NCHUNKS * CSTR_TR], BF16,
                                   name="trA", tag="trA")
                trB = treebuf.tile([P, NCHUNKS * CSTR_TR], BF16,
                                   name="trB", tag="trB")
                h = L // 2
                nc.vector.tensor_add(
                    out=_ap3(trA, 0, CSTR_TR, NCHUNKS, g * h),
                    in0=_ap3(ex, 0, CSTR_EX, NCHUNKS, g * h),
                    in1=_ap3(ex, g * h, CSTR_EX, NCHUNKS, g * h),
                )
                w_, cur, nxt = h, trA, trB
                while w_ > 1:
                    hc = (w_ + 1) // 2
                    hh = w_ - hc
                    nc.vector.tensor_add(
                        out=_ap3(nxt, 0, CSTR_TR, NCHUNKS, g * hh),
                        in0=_ap3(cur, 0, CSTR_TR, NCHUNKS, g * hh),
                        in1=_ap3(cur, g * hc, CSTR_TR, NCHUNKS, g * hh),
                    )
                    if hc > hh:  # odd width: middle column passes through
                        nc.vector.tensor_copy(
                            out=_ap3(nxt, g * hh, CSTR_TR, NCHUNKS, g),
                            in_=_ap3(cur, g * hh, CSTR_TR, NCHUNKS, g),
                        )
                    w_, cur, nxt = hc, nxt, cur
                # s (bf16) sits at offset 0 of each chunk segment of `cur`
                ob = obuf.tile([P, NCHUNKS * g * Lp], I16, name="ob", tag="ob")
                nc.vector.tensor_tensor(
                    out=_ap3(ob, 0, g * Lp, NCHUNKS, g * Lp),
                    in0=_ap3(ex, 0, CSTR_EX, NCHUNKS, g * Lp).bitcast(I16),
                    in1=_ap4(cur, 0, CSTR_TR, NCHUNKS, Lp, g).bitcast(I16),
                    op=mybir.AluOpType.subtract,
                )
                # one DMA for all 8 chunks: DRAM rows c*128+p <- SBUF
                # partition p, cols c*(g*Lp)+j
                od = out_d[:, ob0:ob0 + g * Lp]
                od3 = bass.AP(
                    tensor=od.tensor, offset=od.offset,
                    ap=[[opad, P], [opad * P, NCHUNKS], [1, g * Lp]],
                )
                nc.gpsimd.dma_start(
                    out=od3,
                    in_=_ap3(ob, 0, g * Lp, NCHUNKS, g * Lp),
                )
                ob0 += g * Lp
    nc.compile()
    return nc


# ----------------------------------------------------------------------------
# Entry points
# ----------------------------------------------------------------------------

def run(inputs, trace=False):
    prep = _prep(inputs)
    key = (prep["opad"], tuple(prep["blocks"]))
    nc = _nc_cache.get(key)
    if nc is None:
        print(f"[kernel] opad={prep['opad']} nb={len(prep['blocks'])} "
              f"blocks={prep['blocks']}", file=sys.stderr)
        nc = _build_nc(prep["blocks"], prep["opad"])
        _nc_cache[key] = nc
    in_maps = [
        {
            "W": prep["W"][c],
            "LAMS": prep["LAMS"][c],
            "THZET": prep["thzet"],
        }
        for c in range(NCORES)
    ]
    res = bass_utils.run_bass_kernel_spmd(
        nc, in_maps, core_ids=list(range(NCORES)), trace=trace
    )
    big = np.stack(
        [np.asarray(res.results[c]["O"]) for c in range(NCORES)]
    )  # [8, B, opad] i16
    out = np.ascontiguousarray(
        big[prep["item_shard"], :, prep["item_ocol"]].T
    ).astype(np.float32) * np.float32(K16)
    return out, res


def kernel(**inputs) -> np.ndarray:
    out, _ = run(inputs, trace=False)
    return out


# revision 13
# speedup vs baseline: 1.0884x; 1.0347x over previous
"""Trainium2 Bass kernel for nn_BEMBFlex (within-category log-softmax utility).

Sharding: items dealt by category across 8 cores (categories rank-sorted by
size, rank % 8 -> shard), one SPMD program for all cores. Each core computes
util for all 1024 sessions over its ~1/8 of items, then the within-category
log-softmax locally.

Layout (the key trick vs the previous version): within each block, item
columns are POSITION-MAJOR: col = t * g + q for slot q < g, within-category
position t < L. Consequences:
  - segment sums become a cascade of CONTIGUOUS bf16 adds (DVE 2x mode),
  - the final log-prob op is an int16 TT subtract whose lsc operand is a
    4D AP [part, [256,8], [0,L'], [1,g]] with stride-1 last dim -> 2x mode
    (the old slot-major broadcast AP had a stride-0 last dim -> stuck at 1x),
  - all 8 session-chunks of a block are processed by ONE DVE op via a
    chunk-stride AP dim, amortizing instruction overhead 8x.

Output is in log-bits scale: out_i16 = bits16(ex) - bits16(s), where
ln(x) ~ bits16(x)*ln2/128 - 127*ln2 (biases cancel in the subtraction).
Host multiplies by K16 = ln2/128 during the de-permute.

Blocks are sized g*L <= 512 so one PSUM bank-aligned 512-region holds one
(block, chunk) pair; a [P, 2048] PSUM tile holds 4 chunks and is drained by
a single ScalarE Exp. PE streams 2 matmuls (util + rank-1 lambda) per
region; back-to-back streaming reaches the full-speed PE p-state.
"""

import sys

for _p in ("/opt/trn_rl_repo",):
    if _p not in sys.path:
        sys.path.insert(0, _p)

import ml_dtypes
import numpy as np

import concourse.bass as bass
import concourse.tile as tile
from concourse import bacc, bass_utils, mybir

NUM_USERS = 100000
NUM_ITEMS = 25000
NUM_CATS = 500
LATENT = 64
BATCH = 1024
NCORES = 8
P = 128
NCHUNKS = BATCH // P
REGION = 512          # psum bank-aligned region = one (block, chunk)
CSTR_EX = 512         # ex tile per-chunk stride
CSTR_TR = 256         # tree scratch per-chunk stride
PAD8 = -224.0         # fp8e4m3 pad: hi+lo = -448 -> exp underflows to 0
LN2 = float(np.log(2.0))
K16 = LN2 / (1 << 7)  # bf16-bit -> ln scale

F32 = mybir.dt.float32
BF16 = mybir.dt.bfloat16
I16 = mybir.dt.int16
FP8 = mybir.dt.float8e4

_nc_cache = {}


# ----------------------------------------------------------------------------
# Host-side layout
# ----------------------------------------------------------------------------

def _layout(cat_sizes):
    """Blocks of slots with uniform tree width L, g*L <= REGION.

    Categories sorted by size desc; slot i holds ranks [8i, 8i+8) (one per
    shard). L = max size in the leading slot rounded up to even; g = how many
    slots fit in a 512 region. Lp = max true size in the block (final/out
    cover only positions t < Lp).
    """
    order = np.argsort(-cat_sizes, kind="stable")
    order = order[cat_sizes[order] > 0]
    ncats = len(order)
    nslots = -(-ncats // NCORES)
    slot_max = np.empty(nslots, np.int64)
    for i in range(nslots):
        slot_max[i] = int(cat_sizes[order[i * NCORES]])
    blocks = []  # (slot0, g, L, Lp)
    i = 0
    while i < nslots:
        Lp = int(slot_max[i])
        L = Lp + (Lp & 1)
        L = max(L, 2)
        g = min(REGION // L, nslots - i)
        blocks.append((i, g, L, Lp))
        i += g
    return order, blocks


def _prep(inputs):
    cat = np.asarray(inputs["category_idx"]).astype(np.int64).ravel()
    cat_sizes = np.bincount(cat, minlength=NUM_CATS)
    order, blocks = _layout(cat_sizes)
    nb = len(blocks)

    rank = np.full(NUM_CATS, -1, np.int64)
    rank[order] = np.arange(len(order))

    perm = np.argsort(cat, kind="stable")
    starts = np.searchsorted(cat[perm], np.arange(NUM_CATS))
    within_sorted = np.arange(NUM_ITEMS) - starts[cat[perm]]
    item_within = np.empty(NUM_ITEMS, np.int64)
    item_within[perm] = within_sorted

    # per-slot -> block index, q, and per-block col bases
    nslots = -(-len(order) // NCORES)
    blk_of_slot = np.empty(nslots, np.int64)
    q_of_slot = np.empty(nslots, np.int64)
    g_of_slot = np.empty(nslots, np.int64)
    ob0 = np.empty(nb, np.int64)
    acc = 0
    for b, (s0, g, L, Lp) in enumerate(blocks):
        blk_of_slot[s0:s0 + g] = b
        q_of_slot[s0:s0 + g] = np.arange(g)
        g_of_slot[s0:s0 + g] = g
        ob0[b] = acc
        acc += g * Lp
    opad = acc

    r = rank[cat]
    slot = r // NCORES
    item_shard = r % NCORES
    blk = blk_of_slot[slot]
    item_wcol = blk * REGION + item_within * g_of_slot[slot] + q_of_slot[slot]
    item_ocol = ob0[blk] + item_within * g_of_slot[slot] + q_of_slot[slot]

    alpha = np.ascontiguousarray(np.asarray(inputs["alpha_item"], np.float32))
    obs = np.ascontiguousarray(np.asarray(inputs["item_obs"], np.float32))
    lam = np.asarray(inputs["lambda_item"], np.float32).ravel()

    wpad = nb * REGION
    W = np.zeros((NCORES, 2 * LATENT, wpad), np.float32)
    # lambda as fp8e4m3 hi+lo planes for the DoubleRow rank-1 matmul:
    # block b occupies [b*1024, b*1024+512) = hi, [+512, +1024) = lo
    LAMS8 = np.full((NCORES, 1, nb * 2 * REGION), PAD8, np.float32)
    for s in range(NCORES):
        m = item_shard == s
        cols = item_wcol[m]
        W[s, 0:LATENT, cols] = alpha[m]
        W[s, LATENT:, cols] = obs[m]
        blk_c = cols // REGION
        off_c = cols % REGION
        hi = np.asarray(lam[m].astype(ml_dtypes.float8_e4m3fn), np.float32)
        lo = lam[m] - hi
        LAMS8[s, 0, blk_c * 2 * REGION + off_c] = hi
        LAMS8[s, 0, blk_c * 2 * REGION + REGION + off_c] = lo
    W = W.astype(ml_dtypes.bfloat16)
    LAMS8 = LAMS8.astype(ml_dtypes.float8_e4m3fn)

    uidx = np.asarray(inputs["user_index"]).astype(np.int64).ravel()
    theta = np.asarray(inputs["theta_user"], np.float32)
    zeta = np.asarray(inputs["zeta_user"], np.float32)
    thzet = np.ascontiguousarray(
        np.concatenate([theta[uidx], zeta[uidx]], axis=1).T
    ).astype(ml_dtypes.bfloat16)
    return {
        "blocks": blocks,
        "opad": opad,
        "item_shard": item_shard,
        "item_ocol": item_ocol,
        "W": W,
        "LAMS8": LAMS8,
        "thzet": thzet,
    }


# ----------------------------------------------------------------------------
# Device program
# ----------------------------------------------------------------------------

def _ap3(t2d, off, cstr, n, w):
    """[P, N] tile -> [P, n, w] AP: chunk-stride cstr, packed inner width."""
    ap = t2d[:, :]
    return bass.AP(tensor=ap.tensor, offset=ap.offset + off,
                   ap=[ap.ap[0], [cstr, n], [1, w]])


def _ap4(t2d, off, cstr, n, rep, w):
    """[P, N] tile -> [P, n, rep, w] AP with a step-0 middle dim."""
    ap = t2d[:, :]
    return bass.AP(tensor=ap.tensor, offset=ap.offset + off,
                   ap=[ap.ap[0], [cstr, n], [0, rep], [1, w]])


def _build_nc(blocks, opad):
    nb = len(blocks)
    wpad = nb * REGION
    nc = bacc.Bacc(
        "TRN2",
        debug=False,
        enable_asserts=False,
        target_bir_lowering=False,
        num_devices=NCORES,
    )
    w_d = nc.dram_tensor("W", [2 * LATENT, wpad], BF16, kind="ExternalInput").ap()
    lams_d = nc.dram_tensor("LAMS8", [1, nb * 2 * REGION], FP8,
                            kind="ExternalInput").ap()
    thzet_d = nc.dram_tensor("THZET", [2 * LATENT, BATCH], BF16,
                             kind="ExternalInput").ap()
    out_d = nc.dram_tensor("O", [BATCH, opad], I16, kind="ExternalOutput").ap()

    with tile.TileContext(nc) as tc:
        with (
            tc.tile_pool(name="singles", bufs=1) as singles,
            tc.tile_pool(name="psum_u", bufs=2, space="PSUM") as psum_u,
            tc.tile_pool(name="exbuf", bufs=3) as exbuf,
            tc.tile_pool(name="treebuf", bufs=3) as treebuf,
            tc.tile_pool(name="obuf", bufs=3) as obuf,
        ):
            thzet_sb = singles.tile([2 * LATENT, BATCH], BF16, name="thzet_sb")
            # first 4 chunks land early so the PE can start sooner
            nc.sync.dma_start(out=thzet_sb[:, 0:4 * P], in_=thzet_d[:, 0:4 * P])
            nc.sync.dma_start(out=thzet_sb[:, 4 * P:], in_=thzet_d[:, 4 * P:])
            ones8_sb = singles.tile([1, 2 * P], FP8, name="ones8_sb")
            nc.vector.memset(ones8_sb[:, :], 1.0)
            thze_t = [thzet_sb[:, c * P:(c + 1) * P] for c in range(NCHUNKS)]
            w_sb = singles.tile([2 * LATENT, wpad], BF16, name="w_sb")
            lams_sb = singles.tile([1, nb * 2 * REGION], FP8, name="lams_sb")
            nc.gpsimd.dma_start(out=lams_sb[:, :], in_=lams_d[:, :])
            nc.gpsimd.dma_start(out=w_sb[:, 0:REGION], in_=w_d[:, 0:REGION])
            nc.gpsimd.dma_start(out=w_sb[:, REGION:], in_=w_d[:, REGION:])
            # lhsT for the DoubleRow rank-1: [K=1, ktile=2, M=128] of ones
            ones8_ap = bass.AP(
                tensor=ones8_sb[:, :].tensor, offset=ones8_sb[:, :].offset,
                ap=[ones8_sb[:, :].ap[0], [P, 2], [1, P]],
            )

            ob0 = 0
            for b, (s0, g, L, Lp) in enumerate(blocks):
                wc0 = b * REGION
                cols = g * L
                ex = exbuf.tile([P, NCHUNKS * CSTR_EX], BF16, name="ex", tag="ex")
                # PE + ScalarE: two tiles of 4 chunk-regions each
                lam_rhs_base = lams_sb[:, :]
                for half in range(2):
                    up = psum_u.tile([P, 4 * REGION], F32, name="up", tag="up")
                    for ci in range(4):
                        c = half * 4 + ci
                        nc.tensor.matmul(
                            up[:, ci * REGION:ci * REGION + cols],
                            lhsT=thze_t[c],
                            rhs=w_sb[:, wc0:wc0 + cols],
                            start=True, stop=False,
                        )
                    for ci in range(4):
                        # rank-1 lambda add: fp8 hi+lo DoubleRow (0.5 cyc/row)
                        lam_rhs = bass.AP(
                            tensor=lam_rhs_base.tensor,
                            offset=lam_rhs_base.offset + b * 2 * REGION,
                            ap=[lam_rhs_base.ap[0], [REGION, 2], [1, cols]],
                        )
                        nc.tensor.matmul(
                            up[:, ci * REGION:ci * REGION + cols],
                            lhsT=ones8_ap,
                            rhs=lam_rhs,
                            start=False, stop=True,
                            perf_mode=mybir.MatmulPerfMode.DoubleRow,
                        )
                    # 3D APs skip the (512 - g*L) pad gap of each region
                    nc.scalar.activation(
                        out=_ap3(ex, half * 4 * CSTR_EX, CSTR_EX, 4, cols),
                        in_=_ap3(up, 0, REGION, 4, cols),
                        func=mybir.ActivationFunctionType.Exp,
                    )
                # DVE: cascade of contiguous adds across all 8 chunks at once
                trA = treebuf.tile([P, NCHUNKS * CSTR_TR], BF16,
                                   name="trA", tag="trA")
                trB = treebuf.tile([P, NCHUNKS * CSTR_TR], BF16,
                                   name="trB", tag="trB")
                h = L // 2
                nc.vector.tensor_add(
                    out=_ap3(trA, 0, CSTR_TR, NCHUNKS, g * h),
                    in0=_ap3(ex, 0, CSTR_EX, NCHUNKS, g * h),
                    in1=_ap3(ex, g * h, CSTR_EX, NCHUNKS, g * h),
                )
                w_, cur, nxt = h, trA, trB
                while w_ > 1:
                    hc = (w_ + 1) // 2
                    hh = w_ - hc
                    nc.vector.tensor_add(
                        out=_ap3(nxt, 0, CSTR_TR, NCHUNKS, g * hh),
                        in0=_ap3(cur, 0, CSTR_TR, NCHUNKS, g * hh),
                        in1=_ap3(cur, g * hc, CSTR_TR, NCHUNKS, g * hh),
                    )
                    if hc > hh:  # odd width: middle column passes through
                        nc.vector.tensor_copy(
                            out=_ap3(nxt, g * hh, CSTR_TR, NCHUNKS, g),
                            in_=_ap3(cur, g * hh, CSTR_TR, NCHUNKS, g),
                        )
                    w_, cur, nxt = hc, nxt, cur
                # s (bf16) sits at offset 0 of each chunk segment of `cur`
                ob = obuf.tile([P, NCHUNKS * g * Lp], I16, name="ob", tag="ob")
                nc.vector.tensor_tensor(
                    out=_ap3(ob, 0, g * Lp, NCHUNKS, g * Lp),
                    in0=_ap3(ex, 0, CSTR_EX, NCHUNKS, g * Lp).bitcast(I16),
                    in1=_ap4(cur, 0, CSTR_TR, NCHUNKS, Lp, g).bitcast(I16),
                    op=mybir.AluOpType.subtract,
                )
                # one DMA for all 8 chunks: DRAM rows c*128+p <- SBUF
                # partition p, cols c*(g*Lp)+j
                od = out_d[:, ob0:ob0 + g * Lp]
                od3 = bass.AP(
                    tensor=od.tensor, offset=od.offset,
                    ap=[[opad, P], [opad * P, NCHUNKS], [1, g * Lp]],
                )
                nc.gpsimd.dma_start(
                    out=od3,
                    in_=_ap3(ob, 0, g * Lp, NCHUNKS, g * Lp),
                )
                ob0 += g * Lp
    nc.compile()
    return nc


# ----------------------------------------------------------------------------
# Entry points
# ----------------------------------------------------------------------------

def run(inputs, trace=False):
    prep = _prep(inputs)
    key = (prep["opad"], tuple(prep["blocks"]))
    nc = _nc_cache.get(key)
    if nc is None:
        print(f"[kernel] opad={prep['opad']} nb={len(prep['blocks'])} "
              f"blocks={prep['blocks']}", file=sys.stderr)
        nc = _build_nc(prep["blocks"], prep["opad"])
        _nc_cache[key] = nc
    in_maps = [
        {
            "W": prep["W"][c],
            "LAMS8": prep["LAMS8"][c],
            "THZET": prep["thzet"],
        }
        for c in range(NCORES)
    ]
    res = bass_utils.run_bass_kernel_spmd(
        nc, in_maps, core_ids=list(range(NCORES)), trace=trace
    )
    big = np.stack(
        [np.asarray(res.results[c]["O"]) for c in range(NCORES)]
    )  # [8, B, opad] i16
    out = np.ascontiguousarray(
        big[prep["item_shard"], :, prep["item_ocol"]].T
    ).astype(np.float32) * np.float32(K16)
    return out, res


def kernel(**inputs) -> np.ndarray:
    out, _ = run(inputs, trace=False)
    return out


# revision 17
# speedup vs baseline: 1.1081x; 1.0181x over previous
"""Trainium2 Bass kernel for nn_BEMBFlex (within-category log-softmax utility).

Sharding: items dealt by category across 8 cores (categories rank-sorted by
size, rank % 8 -> shard), one SPMD program for all cores. Each core computes
util for all 1024 sessions over its ~1/8 of items, then the within-category
log-softmax locally.

Layout (the key trick vs the previous version): within each block, item
columns are POSITION-MAJOR: col = t * g + q for slot q < g, within-category
position t < L. Consequences:
  - segment sums become a cascade of CONTIGUOUS bf16 adds (DVE 2x mode),
  - the final log-prob op is an int16 TT subtract whose lsc operand is a
    4D AP [part, [256,8], [0,L'], [1,g]] with stride-1 last dim -> 2x mode
    (the old slot-major broadcast AP had a stride-0 last dim -> stuck at 1x),
  - all 8 session-chunks of a block are processed by ONE DVE op via a
    chunk-stride AP dim, amortizing instruction overhead 8x.

Output is in log-bits scale: out_i16 = bits16(ex) - bits16(s), where
ln(x) ~ bits16(x)*ln2/128 - 127*ln2 (biases cancel in the subtraction).
Host multiplies by K16 = ln2/128 during the de-permute.

Blocks are sized g*L <= 512 so one PSUM bank-aligned 512-region holds one
(block, chunk) pair; a [P, 2048] PSUM tile holds 4 chunks and is drained by
a single ScalarE Exp. PE streams 2 matmuls (util + rank-1 lambda) per
region; back-to-back streaming reaches the full-speed PE p-state.
"""

import sys

for _p in ("/opt/trn_rl_repo",):
    if _p not in sys.path:
        sys.path.insert(0, _p)

import ml_dtypes
import numpy as np

import concourse.bass as bass
import concourse.tile as tile
from concourse import bacc, bass_utils, mybir

NUM_USERS = 100000
NUM_ITEMS = 25000
NUM_CATS = 500
LATENT = 64
BATCH = 1024
NCORES = 8
P = 128
NCHUNKS = BATCH // P
REGION = 512          # psum bank-aligned region = one (block, chunk)
CSTR_EX = 512         # ex tile per-chunk stride
CSTR_TR = 256         # tree scratch per-chunk stride
PAD8 = -224.0         # fp8e4m3 pad: hi+lo = -448 -> exp underflows to 0
LN2 = float(np.log(2.0))
K16 = LN2 / (1 << 7)  # bf16-bit -> ln scale

F32 = mybir.dt.float32
BF16 = mybir.dt.bfloat16
I16 = mybir.dt.int16
FP8 = mybir.dt.float8e4

_nc_cache = {}


# ----------------------------------------------------------------------------
# Host-side layout
# ----------------------------------------------------------------------------

def _layout(cat_sizes):
    """Blocks of slots with uniform tree width L, g*L <= REGION.

    Categories sorted by size desc; slot i holds ranks [8i, 8i+8) (one per
    shard). L = max size in the leading slot rounded up to even; g = how many
    slots fit in a 512 region. Lp = max true size in the block (final/out
    cover only positions t < Lp).
    """
    order = np.argsort(-cat_sizes, kind="stable")
    order = order[cat_sizes[order] > 0]
    ncats = len(order)
    nslots = -(-ncats // NCORES)
    slot_max = np.empty(nslots, np.int64)
    for i in range(nslots):
        slot_max[i] = int(cat_sizes[order[i * NCORES]])
    blocks = []  # (slot0, g, L, Lp)
    i = 0
    while i < nslots:
        Lp = int(slot_max[i])
        L = Lp + (Lp & 1)
        L = max(L, 2)
        g = min(REGION // L, nslots - i)
        blocks.append((i, g, L, Lp))
        i += g
    return order, blocks


def _prep(inputs):
    cat = np.asarray(inputs["category_idx"]).astype(np.int64).ravel()
    cat_sizes = np.bincount(cat, minlength=NUM_CATS)
    order, blocks = _layout(cat_sizes)
    nb = len(blocks)

    rank = np.full(NUM_CATS, -1, np.int64)
    rank[order] = np.arange(len(order))

    perm = np.argsort(cat, kind="stable")
    starts = np.searchsorted(cat[perm], np.arange(NUM_CATS))
    within_sorted = np.arange(NUM_ITEMS) - starts[cat[perm]]
    item_within = np.empty(NUM_ITEMS, np.int64)
    item_within[perm] = within_sorted

    # per-slot -> block index, q, and per-block col bases
    nslots = -(-len(order) // NCORES)
    blk_of_slot = np.empty(nslots, np.int64)
    q_of_slot = np.empty(nslots, np.int64)
    g_of_slot = np.empty(nslots, np.int64)
    ob0 = np.empty(nb, np.int64)
    acc = 0
    for b, (s0, g, L, Lp) in enumerate(blocks):
        blk_of_slot[s0:s0 + g] = b
        q_of_slot[s0:s0 + g] = np.arange(g)
        g_of_slot[s0:s0 + g] = g
        ob0[b] = acc
        acc += g * Lp
    opad = acc

    r = rank[cat]
    slot = r // NCORES
    item_shard = r % NCORES
    blk = blk_of_slot[slot]
    item_wcol = blk * REGION + item_within * g_of_slot[slot] + q_of_slot[slot]
    item_ocol = ob0[blk] + item_within * g_of_slot[slot] + q_of_slot[slot]

    alpha = np.ascontiguousarray(np.asarray(inputs["alpha_item"], np.float32))
    obs = np.ascontiguousarray(np.asarray(inputs["item_obs"], np.float32))
    lam = np.asarray(inputs["lambda_item"], np.float32).ravel()

    wpad = nb * REGION
    W = np.zeros((NCORES, 2 * LATENT, wpad), np.float32)
    # lambda as fp8e4m3 hi+lo planes for the DoubleRow rank-1 matmul:
    # block b occupies [b*1024, b*1024+512) = hi, [+512, +1024) = lo
    LAMS8 = np.full((NCORES, 1, nb * 2 * REGION), PAD8, np.float32)
    for s in range(NCORES):
        m = item_shard == s
        cols = item_wcol[m]
        W[s, 0:LATENT, cols] = alpha[m]
        W[s, LATENT:, cols] = obs[m]
        blk_c = cols // REGION
        off_c = cols % REGION
        hi = np.asarray(lam[m].astype(ml_dtypes.float8_e4m3fn), np.float32)
        lo = lam[m] - hi
        LAMS8[s, 0, blk_c * 2 * REGION + off_c] = hi
        LAMS8[s, 0, blk_c * 2 * REGION + REGION + off_c] = lo
    W = W.astype(ml_dtypes.bfloat16)
    LAMS8 = LAMS8.astype(ml_dtypes.float8_e4m3fn)

    uidx = np.asarray(inputs["user_index"]).astype(np.int64).ravel()
    theta = np.asarray(inputs["theta_user"], np.float32)
    zeta = np.asarray(inputs["zeta_user"], np.float32)
    thzet = np.ascontiguousarray(
        np.concatenate([theta[uidx], zeta[uidx]], axis=1).T
    ).astype(ml_dtypes.bfloat16)
    return {
        "blocks": blocks,
        "opad": opad,
        "item_shard": item_shard,
        "item_ocol": item_ocol,
        "W": W,
        "LAMS8": LAMS8,
        "thzet": thzet,
    }


# ----------------------------------------------------------------------------
# Device program
# ----------------------------------------------------------------------------

def _ap3(t2d, off, cstr, n, w):
    """[P, N] tile -> [P, n, w] AP: chunk-stride cstr, packed inner width."""
    ap = t2d[:, :]
    return bass.AP(tensor=ap.tensor, offset=ap.offset + off,
                   ap=[ap.ap[0], [cstr, n], [1, w]])


def _ap4(t2d, off, cstr, n, rep, w):
    """[P, N] tile -> [P, n, rep, w] AP with a step-0 middle dim."""
    ap = t2d[:, :]
    return bass.AP(tensor=ap.tensor, offset=ap.offset + off,
                   ap=[ap.ap[0], [cstr, n], [0, rep], [1, w]])


def _build_nc(blocks, opad):
    nb = len(blocks)
    wpad = nb * REGION
    nc = bacc.Bacc(
        "TRN2",
        debug=False,
        enable_asserts=False,
        target_bir_lowering=False,
        num_devices=NCORES,
    )
    w_d = nc.dram_tensor("W", [2 * LATENT, wpad], BF16, kind="ExternalInput").ap()
    lams_d = nc.dram_tensor("LAMS8", [1, nb * 2 * REGION], FP8,
                            kind="ExternalInput").ap()
    thzet_d = nc.dram_tensor("THZET", [2 * LATENT, BATCH], BF16,
                             kind="ExternalInput").ap()
    out_d = nc.dram_tensor("O", [BATCH, opad], I16, kind="ExternalOutput").ap()

    # process order: smallest block first (fast first EXP -> early pipeline
    # start), then descending by cols so the drain block is small-ish
    sz = [g * L for (_s, g, L, _p) in blocks]
    order_blocks = [int(np.argmin(sz))] + sorted(
        (b for b in range(nb) if b != int(np.argmin(sz))),
        key=lambda b: -sz[b],
    )

    with tile.TileContext(nc) as tc:
        with (
            tc.tile_pool(name="singles", bufs=1) as singles,
            tc.tile_pool(name="psum_u", bufs=2, space="PSUM") as psum_u,
            tc.tile_pool(name="exbuf", bufs=3) as exbuf,
            tc.tile_pool(name="treebuf", bufs=3) as treebuf,
            tc.tile_pool(name="obuf", bufs=3) as obuf,
        ):
            thzet_sb = singles.tile([2 * LATENT, BATCH], BF16, name="thzet_sb")
            # first 4 chunks land early so the PE can start sooner
            nc.sync.dma_start(out=thzet_sb[:, 0:4 * P], in_=thzet_d[:, 0:4 * P])
            nc.sync.dma_start(out=thzet_sb[:, 4 * P:], in_=thzet_d[:, 4 * P:])
            ones8_sb = singles.tile([1, 2 * P], FP8, name="ones8_sb")
            nc.vector.memset(ones8_sb[:, :], 1.0)
            thze_t = [thzet_sb[:, c * P:(c + 1) * P] for c in range(NCHUNKS)]
            w_sb = singles.tile([2 * LATENT, wpad], BF16, name="w_sb")
            lams_sb = singles.tile([1, nb * 2 * REGION], FP8, name="lams_sb")
            nc.gpsimd.dma_start(out=lams_sb[:, :], in_=lams_d[:, :])
            # W streamed per block in process order (first block lands fast)
            for b in order_blocks:
                c0 = b * REGION
                nc.gpsimd.dma_start(out=w_sb[:, c0:c0 + REGION],
                                    in_=w_d[:, c0:c0 + REGION])
            # lhsT for the DoubleRow rank-1: [K=1, ktile=2, M=128] of ones
            ones8_ap = bass.AP(
                tensor=ones8_sb[:, :].tensor, offset=ones8_sb[:, :].offset,
                ap=[ones8_sb[:, :].ap[0], [P, 2], [1, P]],
            )

            ob0s = []
            acc = 0
            for (s0, g, L, Lp) in blocks:
                ob0s.append(acc)
                acc += g * Lp
            last_b = order_blocks[-1]
            for b in order_blocks:
                (s0, g, L, Lp) = blocks[b]
                ob0 = ob0s[b]
                wc0 = b * REGION
                cols = g * L
                ex = exbuf.tile([P, NCHUNKS * CSTR_EX], BF16, name="ex", tag="ex")
                # PE + ScalarE: two tiles of 4 chunk-regions each
                lam_rhs_base = lams_sb[:, :]
                for half in range(2):
                    up = psum_u.tile([P, 4 * REGION], F32, name="up", tag="up")
                    for ci in range(4):
                        c = half * 4 + ci
                        nc.tensor.matmul(
                            up[:, ci * REGION:ci * REGION + cols],
                            lhsT=thze_t[c],
                            rhs=w_sb[:, wc0:wc0 + cols],
                            start=True, stop=False,
                        )
                    for ci in range(4):
                        # rank-1 lambda add: fp8 hi+lo DoubleRow (0.5 cyc/row)
                        lam_rhs = bass.AP(
                            tensor=lam_rhs_base.tensor,
                            offset=lam_rhs_base.offset + b * 2 * REGION,
                            ap=[lam_rhs_base.ap[0], [REGION, 2], [1, cols]],
                        )
                        nc.tensor.matmul(
                            up[:, ci * REGION:ci * REGION + cols],
                            lhsT=ones8_ap,
                            rhs=lam_rhs,
                            start=False, stop=True,
                            perf_mode=mybir.MatmulPerfMode.DoubleRow,
                        )
                    # 3D APs skip the (512 - g*L) pad gap of each region
                    nc.scalar.activation(
                        out=_ap3(ex, half * 4 * CSTR_EX, CSTR_EX, 4, cols),
                        in_=_ap3(up, 0, REGION, 4, cols),
                        func=mybir.ActivationFunctionType.Exp,
                    )
                # DVE: cascade of contiguous adds, fused across chunk groups;
                # the last processed block splits into halves so its final +
                # out-DMA overlap the second half's exp (shorter drain)
                ob = obuf.tile([P, NCHUNKS * g * Lp], I16, name="ob", tag="ob")
                groups = [(0, 4), (4, 4)] if b == last_b else [(0, NCHUNKS)]
                for (c0, ncr) in groups:
                    trA = treebuf.tile([P, NCHUNKS * CSTR_TR], BF16,
                                       name="trA", tag="trA")
                    trB = treebuf.tile([P, NCHUNKS * CSTR_TR], BF16,
                                       name="trB", tag="trB")
                    exo = c0 * CSTR_EX
                    tro = c0 * CSTR_TR
                    h = L // 2
                    nc.vector.tensor_add(
                        out=_ap3(trA, tro, CSTR_TR, ncr, g * h),
                        in0=_ap3(ex, exo, CSTR_EX, ncr, g * h),
                        in1=_ap3(ex, exo + g * h, CSTR_EX, ncr, g * h),
                    )
                    w_, cur, nxt = h, trA, trB
                    while w_ > 1:
                        hc = (w_ + 1) // 2
                        hh = w_ - hc
                        nc.vector.tensor_add(
                            out=_ap3(nxt, tro, CSTR_TR, ncr, g * hh),
                            in0=_ap3(cur, tro, CSTR_TR, ncr, g * hh),
                            in1=_ap3(cur, tro + g * hc, CSTR_TR, ncr, g * hh),
                        )
                        if hc > hh:  # odd width: middle column passes through
                            nc.vector.tensor_copy(
                                out=_ap3(nxt, tro + g * hh, CSTR_TR, ncr, g),
                                in_=_ap3(cur, tro + g * hh, CSTR_TR, ncr, g),
                            )
                        w_, cur, nxt = hc, nxt, cur
                    # s (bf16) sits at offset 0 of each chunk segment of `cur`
                    nc.vector.tensor_tensor(
                        out=_ap3(ob, c0 * g * Lp, g * Lp, ncr, g * Lp),
                        in0=_ap3(ex, exo, CSTR_EX, ncr, g * Lp).bitcast(I16),
                        in1=_ap4(cur, tro, CSTR_TR, ncr, Lp, g).bitcast(I16),
                        op=mybir.AluOpType.subtract,
                    )
                    # one DMA per group: DRAM rows c*128+p <- SBUF cols
                    od = out_d[:, ob0:ob0 + g * Lp]
                    od3 = bass.AP(
                        tensor=od.tensor,
                        offset=od.offset + c0 * P * opad,
                        ap=[[opad, P], [opad * P, ncr], [1, g * Lp]],
                    )
                    nc.gpsimd.dma_start(
                        out=od3,
                        in_=_ap3(ob, c0 * g * Lp, g * Lp, ncr, g * Lp),
                    )
    nc.compile()
    return nc


# ----------------------------------------------------------------------------
# Entry points
# ----------------------------------------------------------------------------

def run(inputs, trace=False):
    prep = _prep(inputs)
    key = (prep["opad"], tuple(prep["blocks"]))
    nc = _nc_cache.get(key)
    if nc is None:
        print(f"[kernel] opad={prep['opad']} nb={len(prep['blocks'])} "
              f"blocks={prep['blocks']}", file=sys.stderr)
        nc = _build_nc(prep["blocks"], prep["opad"])
        _nc_cache[key] = nc
    in_maps = [
        {
            "W": prep["W"][c],
            "LAMS8": prep["LAMS8"][c],
            "THZET": prep["thzet"],
        }
        for c in range(NCORES)
    ]
    res = bass_utils.run_bass_kernel_spmd(
        nc, in_maps, core_ids=list(range(NCORES)), trace=trace
    )
    big = np.stack(
        [np.asarray(res.results[c]["O"]) for c in range(NCORES)]
    )  # [8, B, opad] i16
    out = np.ascontiguousarray(
        big[prep["item_shard"], :, prep["item_ocol"]].T
    ).astype(np.float32) * np.float32(K16)
    return out, res


def kernel(**inputs) -> np.ndarray:
    out, _ = run(inputs, trace=False)
    return out


# revision 19
# speedup vs baseline: 1.1198x; 1.0106x over previous
"""Trainium2 Bass kernel for nn_BEMBFlex (within-category log-softmax utility).

Sharding: items dealt by category across 8 cores (categories rank-sorted by
size, rank % 8 -> shard), one SPMD program for all cores. Each core computes
util for all 1024 sessions over its ~1/8 of items, then the within-category
log-softmax locally.

Layout (the key trick vs the previous version): within each block, item
columns are POSITION-MAJOR: col = t * g + q for slot q < g, within-category
position t < L. Consequences:
  - segment sums become a cascade of CONTIGUOUS bf16 adds (DVE 2x mode),
  - the final log-prob op is an int16 TT subtract whose lsc operand is a
    4D AP [part, [256,8], [0,L'], [1,g]] with stride-1 last dim -> 2x mode
    (the old slot-major broadcast AP had a stride-0 last dim -> stuck at 1x),
  - all 8 session-chunks of a block are processed by ONE DVE op via a
    chunk-stride AP dim, amortizing instruction overhead 8x.

Output is in log-bits scale: out_i16 = bits16(ex) - bits16(s), where
ln(x) ~ bits16(x)*ln2/128 - 127*ln2 (biases cancel in the subtraction).
Host multiplies by K16 = ln2/128 during the de-permute.

Blocks are sized g*L <= 512 so one PSUM bank-aligned 512-region holds one
(block, chunk) pair; a [P, 2048] PSUM tile holds 4 chunks and is drained by
a single ScalarE Exp. PE streams 2 matmuls (util + rank-1 lambda) per
region; back-to-back streaming reaches the full-speed PE p-state.
"""

import sys

for _p in ("/opt/trn_rl_repo",):
    if _p not in sys.path:
        sys.path.insert(0, _p)

import ml_dtypes
import numpy as np

import concourse.bass as bass
import concourse.tile as tile
from concourse import bacc, bass_utils, mybir

NUM_USERS = 100000
NUM_ITEMS = 25000
NUM_CATS = 500
LATENT = 64
BATCH = 1024
NCORES = 8
P = 128
NCHUNKS = BATCH // P
REGION = 512          # psum bank-aligned region = one (block, chunk)
CSTR_EX = 512         # ex tile per-chunk stride
CSTR_TR = 256         # tree scratch per-chunk stride
PAD8 = -224.0         # fp8e4m3 pad: hi+lo = -448 -> exp underflows to 0
LN2 = float(np.log(2.0))
K16 = LN2 / (1 << 7)  # bf16-bit -> ln scale

F32 = mybir.dt.float32
BF16 = mybir.dt.bfloat16
I16 = mybir.dt.int16
FP8 = mybir.dt.float8e4

_nc_cache = {}


# ----------------------------------------------------------------------------
# Host-side layout
# ----------------------------------------------------------------------------

def _layout(cat_sizes):
    """Blocks of slots with uniform tree width L, g*L <= REGION.

    Categories sorted by size desc; slot i holds ranks [8i, 8i+8) (one per
    shard). L = max size in the leading slot rounded up to even; g = how many
    slots fit in a 512 region. Lp = max true size in the block (final/out
    cover only positions t < Lp).
    """
    order = np.argsort(-cat_sizes, kind="stable")
    order = order[cat_sizes[order] > 0]
    ncats = len(order)
    nslots = -(-ncats // NCORES)
    slot_max = np.empty(nslots, np.int64)
    for i in range(nslots):
        slot_max[i] = int(cat_sizes[order[i * NCORES]])
    blocks = []  # (slot0, g, L, Lp)
    i = 0
    while i < nslots:
        Lp = int(slot_max[i])
        L = Lp + (Lp & 1)
        L = max(L, 2)
        g = min(REGION // L, nslots - i)
        blocks.append((i, g, L, Lp))
        i += g
    return order, blocks


def _prep(inputs):
    cat = np.asarray(inputs["category_idx"]).astype(np.int64).ravel()
    cat_sizes = np.bincount(cat, minlength=NUM_CATS)
    order, blocks = _layout(cat_sizes)
    nb = len(blocks)

    rank = np.full(NUM_CATS, -1, np.int64)
    rank[order] = np.arange(len(order))

    perm = np.argsort(cat, kind="stable")
    starts = np.searchsorted(cat[perm], np.arange(NUM_CATS))
    within_sorted = np.arange(NUM_ITEMS) - starts[cat[perm]]
    item_within = np.empty(NUM_ITEMS, np.int64)
    item_within[perm] = within_sorted

    # per-slot -> block index, q, and per-block col bases
    nslots = -(-len(order) // NCORES)
    blk_of_slot = np.empty(nslots, np.int64)
    q_of_slot = np.empty(nslots, np.int64)
    g_of_slot = np.empty(nslots, np.int64)
    ob0 = np.empty(nb, np.int64)
    acc = 0
    for b, (s0, g, L, Lp) in enumerate(blocks):
        blk_of_slot[s0:s0 + g] = b
        q_of_slot[s0:s0 + g] = np.arange(g)
        g_of_slot[s0:s0 + g] = g
        ob0[b] = acc
        acc += g * Lp
    opad = acc

    r = rank[cat]
    slot = r // NCORES
    item_shard = r % NCORES
    blk = blk_of_slot[slot]
    item_wcol = blk * REGION + item_within * g_of_slot[slot] + q_of_slot[slot]
    item_ocol = ob0[blk] + item_within * g_of_slot[slot] + q_of_slot[slot]

    alpha = np.ascontiguousarray(np.asarray(inputs["alpha_item"], np.float32))
    obs = np.ascontiguousarray(np.asarray(inputs["item_obs"], np.float32))
    lam = np.asarray(inputs["lambda_item"], np.float32).ravel()

    wpad = nb * REGION
    W = np.zeros((NCORES, 2 * LATENT, wpad), np.float32)
    # lambda as fp8e4m3 hi+lo planes for the DoubleRow rank-1 matmul:
    # block b occupies [b*1024, b*1024+512) = hi, [+512, +1024) = lo
    LAMS8 = np.full((NCORES, 1, nb * 2 * REGION), PAD8, np.float32)
    for s in range(NCORES):
        m = item_shard == s
        cols = item_wcol[m]
        W[s, 0:LATENT, cols] = alpha[m]
        W[s, LATENT:, cols] = obs[m]
        blk_c = cols // REGION
        off_c = cols % REGION
        hi = np.asarray(lam[m].astype(ml_dtypes.float8_e4m3fn), np.float32)
        lo = lam[m] - hi
        LAMS8[s, 0, blk_c * 2 * REGION + off_c] = hi
        LAMS8[s, 0, blk_c * 2 * REGION + REGION + off_c] = lo
    W = W.astype(ml_dtypes.bfloat16)
    LAMS8 = LAMS8.astype(ml_dtypes.float8_e4m3fn)

    uidx = np.asarray(inputs["user_index"]).astype(np.int64).ravel()
    theta = np.asarray(inputs["theta_user"], np.float32)
    zeta = np.asarray(inputs["zeta_user"], np.float32)
    thzet = np.ascontiguousarray(
        np.concatenate([theta[uidx], zeta[uidx]], axis=1).T
    ).astype(ml_dtypes.bfloat16)
    return {
        "blocks": blocks,
        "opad": opad,
        "item_shard": item_shard,
        "item_ocol": item_ocol,
        "W": W,
        "LAMS8": LAMS8,
        "thzet": thzet,
    }


# ----------------------------------------------------------------------------
# Device program
# ----------------------------------------------------------------------------

def _ap3(t2d, off, cstr, n, w):
    """[P, N] tile -> [P, n, w] AP: chunk-stride cstr, packed inner width."""
    ap = t2d[:, :]
    return bass.AP(tensor=ap.tensor, offset=ap.offset + off,
                   ap=[ap.ap[0], [cstr, n], [1, w]])


def _ap4(t2d, off, cstr, n, rep, w):
    """[P, N] tile -> [P, n, rep, w] AP with a step-0 middle dim."""
    ap = t2d[:, :]
    return bass.AP(tensor=ap.tensor, offset=ap.offset + off,
                   ap=[ap.ap[0], [cstr, n], [0, rep], [1, w]])


def _build_nc(blocks, opad):
    nb = len(blocks)
    wpad = nb * REGION
    nc = bacc.Bacc(
        "TRN2",
        debug=False,
        enable_asserts=False,
        target_bir_lowering=False,
        num_devices=NCORES,
    )
    w_d = nc.dram_tensor("W", [2 * LATENT, wpad], BF16, kind="ExternalInput").ap()
    lams_d = nc.dram_tensor("LAMS8", [1, nb * 2 * REGION], FP8,
                            kind="ExternalInput").ap()
    thzet_d = nc.dram_tensor("THZET", [2 * LATENT, BATCH], BF16,
                             kind="ExternalInput").ap()
    out_d = nc.dram_tensor("O", [BATCH, opad], I16, kind="ExternalOutput").ap()

    # process order: block 0 first (its W slice is one small leading DMA),
    # then descending by cols so the drain block is the smallest
    sz = [g * L for (_s, g, L, _p) in blocks]
    order_blocks = [0] + sorted(range(1, nb), key=lambda b: -sz[b])

    with tile.TileContext(nc) as tc:
        with (
            tc.tile_pool(name="singles", bufs=1) as singles,
            tc.tile_pool(name="psum_u", bufs=2, space="PSUM") as psum_u,
            tc.tile_pool(name="exbuf", bufs=3) as exbuf,
            tc.tile_pool(name="treebuf", bufs=3) as treebuf,
            tc.tile_pool(name="obuf", bufs=3) as obuf,
        ):
            thzet_sb = singles.tile([2 * LATENT, BATCH], BF16, name="thzet_sb")
            # first 4 chunks land early so the PE can start sooner
            nc.sync.dma_start(out=thzet_sb[:, 0:4 * P], in_=thzet_d[:, 0:4 * P])
            nc.sync.dma_start(out=thzet_sb[:, 4 * P:], in_=thzet_d[:, 4 * P:])
            ones8_sb = singles.tile([1, 2 * P], FP8, name="ones8_sb")
            nc.vector.memset(ones8_sb[:, :], 1.0)
            thze_t = [thzet_sb[:, c * P:(c + 1) * P] for c in range(NCHUNKS)]
            w_sb = singles.tile([2 * LATENT, wpad], BF16, name="w_sb")
            lams_sb = singles.tile([1, nb * 2 * REGION], FP8, name="lams_sb")
            nc.gpsimd.dma_start(out=lams_sb[:, :], in_=lams_d[:, :])
            nc.gpsimd.dma_start(out=w_sb[:, 0:REGION], in_=w_d[:, 0:REGION])
            nc.gpsimd.dma_start(out=w_sb[:, REGION:], in_=w_d[:, REGION:])
            # lhsT for the DoubleRow rank-1: [K=1, ktile=2, M=128] of ones
            ones8_ap = bass.AP(
                tensor=ones8_sb[:, :].tensor, offset=ones8_sb[:, :].offset,
                ap=[ones8_sb[:, :].ap[0], [P, 2], [1, P]],
            )

            ob0s = []
            acc = 0
            for (s0, g, L, Lp) in blocks:
                ob0s.append(acc)
                acc += g * Lp
            last_b = order_blocks[-1]
            for b in order_blocks:
                (s0, g, L, Lp) = blocks[b]
                ob0 = ob0s[b]
                wc0 = b * REGION
                cols = g * L
                ex = exbuf.tile([P, NCHUNKS * CSTR_EX], BF16, name="ex", tag="ex")
                # PE + ScalarE: two tiles of 4 chunk-regions each
                lam_rhs_base = lams_sb[:, :]
                for half in range(2):
                    up = psum_u.tile([P, 4 * REGION], F32, name="up", tag="up")
                    for ci in range(4):
                        c = half * 4 + ci
                        nc.tensor.matmul(
                            up[:, ci * REGION:ci * REGION + cols],
                            lhsT=thze_t[c],
                            rhs=w_sb[:, wc0:wc0 + cols],
                            start=True, stop=False,
                        )
                    for ci in range(4):
                        # rank-1 lambda add: fp8 hi+lo DoubleRow (0.5 cyc/row)
                        lam_rhs = bass.AP(
                            tensor=lam_rhs_base.tensor,
                            offset=lam_rhs_base.offset + b * 2 * REGION,
                            ap=[lam_rhs_base.ap[0], [REGION, 2], [1, cols]],
                        )
                        nc.tensor.matmul(
                            up[:, ci * REGION:ci * REGION + cols],
                            lhsT=ones8_ap,
                            rhs=lam_rhs,
                            start=False, stop=True,
                            perf_mode=mybir.MatmulPerfMode.DoubleRow,
                        )
                    # 3D APs skip the (512 - g*L) pad gap of each region
                    nc.scalar.activation(
                        out=_ap3(ex, half * 4 * CSTR_EX, CSTR_EX, 4, cols),
                        in_=_ap3(up, 0, REGION, 4, cols),
                        func=mybir.ActivationFunctionType.Exp,
                    )
                # DVE: cascade of contiguous adds, fused across chunk groups;
                # the last processed block splits into halves so its final +
                # out-DMA overlap the second half's exp (shorter drain)
                ob = obuf.tile([P, NCHUNKS * g * Lp], I16, name="ob", tag="ob")
                groups = [(0, 4), (4, 4)] if b == last_b else [(0, NCHUNKS)]
                for (c0, ncr) in groups:
                    trA = treebuf.tile([P, NCHUNKS * CSTR_TR], BF16,
                                       name="trA", tag="trA")
                    trB = treebuf.tile([P, NCHUNKS * CSTR_TR], BF16,
                                       name="trB", tag="trB")
                    exo = c0 * CSTR_EX
                    tro = c0 * CSTR_TR
                    h = L // 2
                    nc.vector.tensor_add(
                        out=_ap3(trA, tro, CSTR_TR, ncr, g * h),
                        in0=_ap3(ex, exo, CSTR_EX, ncr, g * h),
                        in1=_ap3(ex, exo + g * h, CSTR_EX, ncr, g * h),
                    )
                    w_, cur, nxt = h, trA, trB
                    while w_ > 1:
                        hc = (w_ + 1) // 2
                        hh = w_ - hc
                        nc.vector.tensor_add(
                            out=_ap3(nxt, tro, CSTR_TR, ncr, g * hh),
                            in0=_ap3(cur, tro, CSTR_TR, ncr, g * hh),
                            in1=_ap3(cur, tro + g * hc, CSTR_TR, ncr, g * hh),
                        )
                        if hc > hh:  # odd width: middle column passes through
                            nc.vector.tensor_copy(
                                out=_ap3(nxt, tro + g * hh, CSTR_TR, ncr, g),
                                in_=_ap3(cur, tro + g * hh, CSTR_TR, ncr, g),
                            )
                        w_, cur, nxt = hc, nxt, cur
                    # s (bf16) sits at offset 0 of each chunk segment of `cur`
                    nc.vector.tensor_tensor(
                        out=_ap3(ob, c0 * g * Lp, g * Lp, ncr, g * Lp),
                        in0=_ap3(ex, exo, CSTR_EX, ncr, g * Lp).bitcast(I16),
                        in1=_ap4(cur, tro, CSTR_TR, ncr, Lp, g).bitcast(I16),
                        op=mybir.AluOpType.subtract,
                    )
                    # one DMA per group: DRAM rows c*128+p <- SBUF cols
                    od = out_d[:, ob0:ob0 + g * Lp]
                    od3 = bass.AP(
                        tensor=od.tensor,
                        offset=od.offset + c0 * P * opad,
                        ap=[[opad, P], [opad * P, ncr], [1, g * Lp]],
                    )
                    nc.gpsimd.dma_start(
                        out=od3,
                        in_=_ap3(ob, c0 * g * Lp, g * Lp, ncr, g * Lp),
                    )
    nc.compile()
    return nc


# ----------------------------------------------------------------------------
# Entry points
# ----------------------------------------------------------------------------

def run(inputs, trace=False):
    prep = _prep(inputs)
    key = (prep["opad"], tuple(prep["blocks"]))
    nc = _nc_cache.get(key)
    if nc is None:
        print(f"[kernel] opad={prep['opad']} nb={len(prep['blocks'])} "
              f"blocks={prep['blocks']}", file=sys.stderr)
        nc = _build_nc(prep["blocks"], prep["opad"])
        _nc_cache[key] = nc
    in_maps = [
        {
            "W": prep["W"][c],
            "LAMS8": prep["LAMS8"][c],
            "THZET": prep["thzet"],
        }
        for c in range(NCORES)
    ]
    res = bass_utils.run_bass_kernel_spmd(
        nc, in_maps, core_ids=list(range(NCORES)), trace=trace
    )
    big = np.stack(
        [np.asarray(res.results[c]["O"]) for c in range(NCORES)]
    )  # [8, B, opad] i16
    out = np.ascontiguousarray(
        big[prep["item_shard"], :, prep["item_ocol"]].T
    ).astype(np.float32) * np.float32(K16)
    return out, res


def kernel(**inputs) -> np.ndarray:
    out, _ = run(inputs, trace=False)
    return out


# revision 22
# speedup vs baseline: 1.1533x; 1.0300x over previous
"""Trainium2 Bass kernel for nn_BEMBFlex (within-category log-softmax utility).

Sharding: items dealt by category across 8 cores (categories rank-sorted by
size, rank % 8 -> shard), one SPMD program for all cores. Each core computes
util for all 1024 sessions over its ~1/8 of items, then the within-category
log-softmax locally.

Layout (the key trick vs the previous version): within each block, item
columns are POSITION-MAJOR: col = t * g + q for slot q < g, within-category
position t < L. Consequences:
  - segment sums become a cascade of CONTIGUOUS bf16 adds (DVE 2x mode),
  - the final log-prob op is an int16 TT subtract whose lsc operand is a
    4D AP [part, [256,8], [0,L'], [1,g]] with stride-1 last dim -> 2x mode
    (the old slot-major broadcast AP had a stride-0 last dim -> stuck at 1x),
  - all 8 session-chunks of a block are processed by ONE DVE op via a
    chunk-stride AP dim, amortizing instruction overhead 8x.

Output is in log-bits scale: out_i16 = bits16(ex) - bits16(s), where
ln(x) ~ bits16(x)*ln2/128 - 127*ln2 (biases cancel in the subtraction).
Host multiplies by K16 = ln2/128 during the de-permute.

Blocks are sized g*L <= 512 so one PSUM bank-aligned 512-region holds one
(block, chunk) pair; a [P, 2048] PSUM tile holds 4 chunks and is drained by
a single ScalarE Exp. PE streams 2 matmuls (util + rank-1 lambda) per
region; back-to-back streaming reaches the full-speed PE p-state.
"""

import sys

for _p in ("/opt/trn_rl_repo",):
    if _p not in sys.path:
        sys.path.insert(0, _p)

import ml_dtypes
import numpy as np

import concourse.bass as bass
import concourse.tile as tile
from concourse import bacc, bass_utils, mybir

NUM_USERS = 100000
NUM_ITEMS = 25000
NUM_CATS = 500
LATENT = 64
BATCH = 1024
NCORES = 8
P = 128
NCHUNKS = BATCH // P
REGION = 512          # psum bank-aligned region = one (block, chunk)
CSTR_EX = 512         # ex tile per-chunk stride
CSTR_TR = 256         # tree scratch per-chunk stride
PAD8 = -224.0         # fp8e4m3 pad: hi+lo = -448 -> exp underflows to 0
LN2 = float(np.log(2.0))
K16 = LN2 / (1 << 7)  # bf16-bit -> ln scale

F32 = mybir.dt.float32
BF16 = mybir.dt.bfloat16
I16 = mybir.dt.int16
FP8 = mybir.dt.float8e4

_nc_cache = {}


# ----------------------------------------------------------------------------
# Host-side layout
# ----------------------------------------------------------------------------

def _layout(cat_sizes):
    """Blocks of slots with uniform tree width L, g*L <= REGION.

    Categories sorted by size desc; slot i holds ranks [8i, 8i+8) (one per
    shard). L = max size in the leading slot rounded up to even; g = how many
    slots fit in a 512 region. Lp = max true size in the block (final/out
    cover only positions t < Lp).
    """
    order = np.argsort(-cat_sizes, kind="stable")
    order = order[cat_sizes[order] > 0]
    ncats = len(order)
    nslots = -(-ncats // NCORES)
    slot_max = np.empty(nslots, np.int64)
    for i in range(nslots):
        slot_max[i] = int(cat_sizes[order[i * NCORES]])
    blocks = []  # (slot0, g, L, Lp)
    i = 0
    while i < nslots:
        Lp = int(slot_max[i])
        L = Lp + (Lp & 1)
        L = max(L, 2)
        g = min(REGION // L, nslots - i)
        blocks.append((i, g, L, Lp))
        i += g
    return order, blocks


def _prep(inputs):
    cat = np.asarray(inputs["category_idx"]).astype(np.int64).ravel()
    cat_sizes = np.bincount(cat, minlength=NUM_CATS)
    order, blocks = _layout(cat_sizes)
    nb = len(blocks)

    rank = np.full(NUM_CATS, -1, np.int64)
    rank[order] = np.arange(len(order))

    perm = np.argsort(cat, kind="stable")
    starts = np.searchsorted(cat[perm], np.arange(NUM_CATS))
    within_sorted = np.arange(NUM_ITEMS) - starts[cat[perm]]
    item_within = np.empty(NUM_ITEMS, np.int64)
    item_within[perm] = within_sorted

    # per-slot -> block index, q, and per-block col bases
    nslots = -(-len(order) // NCORES)
    blk_of_slot = np.empty(nslots, np.int64)
    q_of_slot = np.empty(nslots, np.int64)
    g_of_slot = np.empty(nslots, np.int64)
    ob0 = np.empty(nb, np.int64)
    acc = 0
    for b, (s0, g, L, Lp) in enumerate(blocks):
        blk_of_slot[s0:s0 + g] = b
        q_of_slot[s0:s0 + g] = np.arange(g)
        g_of_slot[s0:s0 + g] = g
        ob0[b] = acc
        acc += g * Lp
    opad = acc

    r = rank[cat]
    slot = r // NCORES
    item_shard = r % NCORES
    blk = blk_of_slot[slot]
    item_wcol = blk * REGION + item_within * g_of_slot[slot] + q_of_slot[slot]
    item_ocol = ob0[blk] + item_within * g_of_slot[slot] + q_of_slot[slot]

    alpha = np.ascontiguousarray(np.asarray(inputs["alpha_item"], np.float32))
    obs = np.ascontiguousarray(np.asarray(inputs["item_obs"], np.float32))
    lam = np.asarray(inputs["lambda_item"], np.float32).ravel()

    wpad = nb * REGION
    W = np.zeros((NCORES, 2 * LATENT, wpad), np.float32)
    # lambda as fp8e4m3 hi+lo planes for the DoubleRow rank-1 matmul:
    # block b occupies [b*1024, b*1024+512) = hi, [+512, +1024) = lo
    LAMS8 = np.full((NCORES, 1, nb * 2 * REGION), PAD8, np.float32)
    for s in range(NCORES):
        m = item_shard == s
        cols = item_wcol[m]
        W[s, 0:LATENT, cols] = alpha[m]
        W[s, LATENT:, cols] = obs[m]
        blk_c = cols // REGION
        off_c = cols % REGION
        hi = np.asarray(lam[m].astype(ml_dtypes.float8_e4m3fn), np.float32)
        lo = lam[m] - hi
        LAMS8[s, 0, blk_c * 2 * REGION + off_c] = hi
        LAMS8[s, 0, blk_c * 2 * REGION + REGION + off_c] = lo
    W = W.astype(ml_dtypes.bfloat16)
    LAMS8 = LAMS8.astype(ml_dtypes.float8_e4m3fn)

    uidx = np.asarray(inputs["user_index"]).astype(np.int64).ravel()
    theta = np.asarray(inputs["theta_user"], np.float32)
    zeta = np.asarray(inputs["zeta_user"], np.float32)
    thzet = np.ascontiguousarray(
        np.concatenate([theta[uidx], zeta[uidx]], axis=1).T
    ).astype(ml_dtypes.bfloat16)
    return {
        "blocks": blocks,
        "opad": opad,
        "item_shard": item_shard,
        "item_ocol": item_ocol,
        "W": W,
        "LAMS8": LAMS8,
        "thzet": thzet,
    }


# ----------------------------------------------------------------------------
# Device program
# ----------------------------------------------------------------------------

def _ap3(t2d, off, cstr, n, w):
    """[P, N] tile -> [P, n, w] AP: chunk-stride cstr, packed inner width."""
    ap = t2d[:, :]
    return bass.AP(tensor=ap.tensor, offset=ap.offset + off,
                   ap=[ap.ap[0], [cstr, n], [1, w]])


def _ap4(t2d, off, cstr, n, rep, w):
    """[P, N] tile -> [P, n, rep, w] AP with a step-0 middle dim."""
    ap = t2d[:, :]
    return bass.AP(tensor=ap.tensor, offset=ap.offset + off,
                   ap=[ap.ap[0], [cstr, n], [0, rep], [1, w]])


def _build_nc(blocks, opad):
    nb = len(blocks)
    wpad = nb * REGION
    nc = bacc.Bacc(
        "TRN2",
        debug=False,
        enable_asserts=False,
        target_bir_lowering=False,
        num_devices=NCORES,
    )
    w_d = nc.dram_tensor("W", [2 * LATENT, wpad], BF16, kind="ExternalInput").ap()
    lams_d = nc.dram_tensor("LAMS8", [1, nb * 2 * REGION], FP8,
                            kind="ExternalInput").ap()
    thzet_d = nc.dram_tensor("THZET", [2 * LATENT, BATCH], BF16,
                             kind="ExternalInput").ap()
    out_d = nc.dram_tensor("O", [BATCH, opad], I16, kind="ExternalOutput").ap()

    # process order: block 0 first (its W slice is one small leading DMA),
    # then descending by cols so the drain block is the smallest
    sz = [g * L for (_s, g, L, _p) in blocks]
    order_blocks = [0] + sorted(range(1, nb), key=lambda b: -sz[b])

    with tile.TileContext(nc) as tc:
        with (
            tc.tile_pool(name="singles", bufs=1) as singles,
            tc.tile_pool(name="psum_u", bufs=2, space="PSUM") as psum_u,
            tc.tile_pool(name="exbuf", bufs=3) as exbuf,
            tc.tile_pool(name="treebuf", bufs=3) as treebuf,
            tc.tile_pool(name="obuf", bufs=3) as obuf,
        ):
            thzet_sb = singles.tile([2 * LATENT, BATCH], BF16, name="thzet_sb")
            # first 4 chunks land early so the PE can start sooner
            nc.sync.dma_start(out=thzet_sb[:, 0:4 * P], in_=thzet_d[:, 0:4 * P])
            nc.sync.dma_start(out=thzet_sb[:, 4 * P:], in_=thzet_d[:, 4 * P:])
            ones8_sb = singles.tile([1, 2 * P], FP8, name="ones8_sb")
            nc.vector.memset(ones8_sb[:, :], 1.0)
            thze_t = [thzet_sb[:, c * P:(c + 1) * P] for c in range(NCHUNKS)]
            w_sb = singles.tile([2 * LATENT, wpad], BF16, name="w_sb")
            lams_sb = singles.tile([1, nb * 2 * REGION], FP8, name="lams_sb")
            nc.gpsimd.dma_start(out=w_sb[:, 0:REGION], in_=w_d[:, 0:REGION])
            nc.gpsimd.dma_start(out=lams_sb[:, :], in_=lams_d[:, :])
            nc.gpsimd.dma_start(out=w_sb[:, REGION:], in_=w_d[:, REGION:])
            # lhsT for the DoubleRow rank-1: [K=1, ktile=2, M=128] of ones
            ones8_ap = bass.AP(
                tensor=ones8_sb[:, :].tensor, offset=ones8_sb[:, :].offset,
                ap=[ones8_sb[:, :].ap[0], [P, 2], [1, P]],
            )

            ob0s = []
            acc = 0
            for (s0, g, L, Lp) in blocks:
                ob0s.append(acc)
                acc += g * Lp
            split_bs = set(order_blocks[-3:])
            for b in order_blocks:
                (s0, g, L, Lp) = blocks[b]
                ob0 = ob0s[b]
                wc0 = b * REGION
                cols = g * L
                ex = exbuf.tile([P, NCHUNKS * CSTR_EX], BF16, name="ex", tag="ex")
                # PE + ScalarE: two tiles of 4 chunk-regions each
                lam_rhs_base = lams_sb[:, :]
                for half in range(2):
                    up = psum_u.tile([P, 4 * REGION], F32, name="up", tag="up")
                    for ci in range(4):
                        c = half * 4 + ci
                        nc.tensor.matmul(
                            up[:, ci * REGION:ci * REGION + cols],
                            lhsT=thze_t[c],
                            rhs=w_sb[:, wc0:wc0 + cols],
                            start=True, stop=False,
                        )
                    for ci in range(4):
                        # rank-1 lambda add: fp8 hi+lo DoubleRow (0.5 cyc/row)
                        lam_rhs = bass.AP(
                            tensor=lam_rhs_base.tensor,
                            offset=lam_rhs_base.offset + b * 2 * REGION,
                            ap=[lam_rhs_base.ap[0], [REGION, 2], [1, cols]],
                        )
                        nc.tensor.matmul(
                            up[:, ci * REGION:ci * REGION + cols],
                            lhsT=ones8_ap,
                            rhs=lam_rhs,
                            start=False, stop=True,
                            perf_mode=mybir.MatmulPerfMode.DoubleRow,
                        )
                    # 3D APs skip the (512 - g*L) pad gap of each region
                    nc.scalar.activation(
                        out=_ap3(ex, half * 4 * CSTR_EX, CSTR_EX, 4, cols),
                        in_=_ap3(up, 0, REGION, 4, cols),
                        func=mybir.ActivationFunctionType.Exp,
                    )
                # DVE: cascade of contiguous adds, fused across chunk groups;
                # the last processed block splits into halves so its final +
                # out-DMA overlap the second half's exp (shorter drain)
                ob = obuf.tile([P, NCHUNKS * g * Lp], I16, name="ob", tag="ob")
                groups = [(0, 4), (4, 4)] if b in split_bs else [(0, NCHUNKS)]
                for (c0, ncr) in groups:
                    trA = treebuf.tile([P, NCHUNKS * CSTR_TR], BF16,
                                       name="trA", tag="trA")
                    trB = treebuf.tile([P, NCHUNKS * CSTR_TR], BF16,
                                       name="trB", tag="trB")
                    exo = c0 * CSTR_EX
                    tro = c0 * CSTR_TR
                    h = L // 2
                    nc.vector.tensor_add(
                        out=_ap3(trA, tro, CSTR_TR, ncr, g * h),
                        in0=_ap3(ex, exo, CSTR_EX, ncr, g * h),
                        in1=_ap3(ex, exo + g * h, CSTR_EX, ncr, g * h),
                    )
                    w_, cur, nxt = h, trA, trB
                    while w_ > 1:
                        hc = (w_ + 1) // 2
                        hh = w_ - hc
                        nc.vector.tensor_add(
                            out=_ap3(nxt, tro, CSTR_TR, ncr, g * hh),
                            in0=_ap3(cur, tro, CSTR_TR, ncr, g * hh),
                            in1=_ap3(cur, tro + g * hc, CSTR_TR, ncr, g * hh),
                        )
                        if hc > hh:  # odd width: middle column passes through
                            nc.vector.tensor_copy(
                                out=_ap3(nxt, tro + g * hh, CSTR_TR, ncr, g),
                                in_=_ap3(cur, tro + g * hh, CSTR_TR, ncr, g),
                            )
                        w_, cur, nxt = hc, nxt, cur
                    # s (bf16) sits at offset 0 of each chunk segment of `cur`
                    nc.vector.tensor_tensor(
                        out=_ap3(ob, c0 * g * Lp, g * Lp, ncr, g * Lp),
                        in0=_ap3(ex, exo, CSTR_EX, ncr, g * Lp).bitcast(I16),
                        in1=_ap4(cur, tro, CSTR_TR, ncr, Lp, g).bitcast(I16),
                        op=mybir.AluOpType.subtract,
                    )
                    # one DMA per group: DRAM rows c*128+p <- SBUF cols
                    od = out_d[:, ob0:ob0 + g * Lp]
                    od3 = bass.AP(
                        tensor=od.tensor,
                        offset=od.offset + c0 * P * opad,
                        ap=[[opad, P], [opad * P, ncr], [1, g * Lp]],
                    )
                    nc.gpsimd.dma_start(
                        out=od3,
                        in_=_ap3(ob, c0 * g * Lp, g * Lp, ncr, g * Lp),
                    )
    nc.compile()
    return nc


# ----------------------------------------------------------------------------
# Entry points
# ----------------------------------------------------------------------------

def run(inputs, trace=False):
    prep = _prep(inputs)
    key = (prep["opad"], tuple(prep["blocks"]))
    nc = _nc_cache.get(key)
    if nc is None:
        print(f"[kernel] opad={prep['opad']} nb={len(prep['blocks'])} "
              f"blocks={prep['blocks']}", file=sys.stderr)
        nc = _build_nc(prep["blocks"], prep["opad"])
        _nc_cache[key] = nc
    in_maps = [
        {
            "W": prep["W"][c],
            "LAMS8": prep["LAMS8"][c],
            "THZET": prep["thzet"],
        }
        for c in range(NCORES)
    ]
    res = bass_utils.run_bass_kernel_spmd(
        nc, in_maps, core_ids=list(range(NCORES)), trace=trace
    )
    big = np.stack(
        [np.asarray(res.results[c]["O"]) for c in range(NCORES)]
    )  # [8, B, opad] i16
    out = np.ascontiguousarray(
        big[prep["item_shard"], :, prep["item_ocol"]].T
    ).astype(np.float32) * np.float32(K16)
    return out, res


def kernel(**inputs) -> np.ndarray:
    out, _ = run(inputs, trace=False)
    return out
